# revision 1
# baseline (speedup 1.0000x reference)
"""Bahdanau-style additive attention on 8 TRN2 NeuronCores.

  hidden = tanh(q @ Wq + k @ Wk)        (B, L, H)
  scores = hidden @ v_param             (B, L)
  attn   = softmax(scores, axis=-1)
  out    = attn @ v                     (B, D)

Sharding: data-parallel over batch — 4 batches per core (B=32, 8 cores).

Per-core device pipeline:

  W1  preT[H, L]   = Wk.T @ kT          stationary=Wk, moving=host-transposed k
  ACT hiddenT      = tanh(preT + qWq_b) per-partition bias
  W2  scores[L, 1] = hiddenT.T @ vp     stationary=hidden chunk -> score COLUMNS
  ACT w = exp(scores)                   no max-subtraction (|scores| << 88)
  W3  acc[1, D+1]  = w.T @ [v | 1]      stationary=w column (float32r), the
                                        ones column gives the softmax
                                        denominator for free
  host: out = acc[:D] / acc[D]

MODE="hilo": W1/W2 run as bf16 hi+lo split pairs (x = hi + lo exactly to
~2^-17), 3 matmuls each with the negligible lo*lo term dropped — fp32-grade
precision at bf16 PE speed, same DMA bytes as fp32.
MODE="f32r": W1/W2 in float32r (TF32-like, ~11-bit-mantissa RNE) — fewer
PE instructions, ~1e-3 relative error.
"""

import ml_dtypes
import numpy as np

import concourse.bass as bass
import concourse.mybir as mybir
from concourse.tile import TileContext

B, L, D, H = 32, 8192, 128, 128
NCORES = 8
BPC = B // NCORES  # batches per core
CHUNK = 512  # L positions per W1/tanh chunk (psum bank limit)
NCH = L // CHUNK  # 16 chunks per batch
KTILE = 2048  # L positions per kT DMA tile
KCH = KTILE // CHUNK  # W1 chunks per kT tile
SUB = 128  # L positions per W2/W3 sub-chunk (stationary width)
NSUB = CHUNK // SUB  # 4
DV = 132  # v row: 128 data + ones col + 3 pad
VT_COLS = 16  # W3 sub-chunks per v SBUF tile
NVT = L // (SUB * VT_COLS)  # 4 v tiles per batch

MODE = "hilo"  # "hilo" | "f32r"

F32 = mybir.dt.float32
F32R = mybir.dt.float32r
BF16 = mybir.dt.bfloat16
ACTF = mybir.ActivationFunctionType
ALU = mybir.AluOpType

_CACHE = {}


def _split_excess_waits(nc, max_waits=1):
    """walrus in this env accepts at most one sync-wait per instruction;
    move extras onto InstNoOps placed just before (same engine, in order)."""
    for fn in nc.m.functions:
        for bb in fn.blocks:
            insts = list(bb.instructions)
            new_insts = []
            for ins in insts:
                si = ins.sync_info
                waits = list(si.on_wait) if si and si.on_wait else []
                if len(waits) > max_waits:
                    extra, keep = waits[:-max_waits], waits[-max_waits:]
                    for g0 in range(0, len(extra), max_waits):
                        pre = mybir.InstNoOp(
                            name=f"{ins.name}-waitsplit{g0}",
                            engine=ins.engine,
                            ins=[],
                            outs=[],
                            sync_info=mybir.SyncInfo(
                                on_wait=extra[g0 : g0 + max_waits], on_update=[]
                            ),
                        )
                        nc.register_instruction(pre, overwrite=True)
                        new_insts.append(pre)
                    ins.sync_info = mybir.SyncInfo(
                        on_wait=keep, on_update=list(si.on_update or [])
                    )
                new_insts.append(ins)
            if len(new_insts) != len(insts):
                bb.instructions[:] = new_insts


def build_nc(mode=MODE):
    nc = bass.Bass("TRN2")
    hilo = mode == "hilo"

    if hilo:
        kh_in = nc.dram_tensor("kh", [BPC, D, L], BF16, kind="ExternalInput")
        kl_in = nc.dram_tensor("kl", [BPC, D, L], BF16, kind="ExternalInput")
        # packed consts: cols 0:4 qwq (f32), 4:68 wkh, 68:132 wkl (bf16 pairs),
        # 132 vph|vpl interleaved as one f32 col
        cst_in = nc.dram_tensor("cst", [128, 133], F32, kind="ExternalInput")
    else:
        kT_in = nc.dram_tensor("kT", [BPC, D, L], F32R, kind="ExternalInput")
        wk_in = nc.dram_tensor("wk", [D, H], F32R, kind="ExternalInput")
        vp_in = nc.dram_tensor("vp", [H, 4], F32R, kind="ExternalInput")
    v_in = nc.dram_tensor("vv", [BPC, NVT, SUB, VT_COLS * DV], F32R, kind="ExternalInput")
    if not hilo:
        qwq_in = nc.dram_tensor("qwq", [H, BPC], F32, kind="ExternalInput")
    out_d = nc.dram_tensor("out", [1, BPC * DV], F32, kind="ExternalOutput")

    with TileContext(nc) as tc:
        with (
            tc.tile_pool(name="const", bufs=1) as cpool,
            tc.tile_pool(name="kp", bufs=8) as kpool,
            tc.tile_pool(name="vp_", bufs=2 * NVT) as vpool,
            tc.tile_pool(name="hp", bufs=4) as hpool,
            tc.tile_pool(name="wp", bufs=2) as wpool,
            tc.tile_pool(name="ob", bufs=1) as opool,
            tc.tile_pool(name="pre", bufs=2, space="PSUM") as pre_pool,
            tc.tile_pool(name="sps", bufs=2, space="PSUM") as s_pool,
            tc.tile_pool(name="ops", bufs=2, space="PSUM") as o_pool,
        ):
            # HAM warm-up on zeroed tiles: needs no DMA, so the PE clock
            # gate lifts during the Tile preamble / first transfers.
            zwarm = cpool.tile([128, 512], BF16)
            nc.gpsimd.memset(zwarm[:], 0.0)
            warm_ps = pre_pool.tile([H, CHUNK], F32, tag="pre")
            for _ in range(16):
                nc.tensor.matmul(
                    warm_ps[:, :512], zwarm[:, :128], zwarm[:], start=True, stop=True
                )

            if hilo:
                cst = cpool.tile([128, 133], F32)
                nc.sync.dma_start(cst[:], cst_in[:])
                qwq = cst[:, 0:4]
                wkh = cst[:, 4:68].bitcast(BF16)
                wkl = cst[:, 68:132].bitcast(BF16)
                vph = cst[:, 132:133].bitcast(BF16)[:, 0:1]
                vpl = cst[:, 132:133].bitcast(BF16)[:, 1:2]
            else:
                qwq = cpool.tile([H, BPC], F32)
                nc.sync.dma_start(qwq[:], qwq_in[:])
                wk = cpool.tile([D, H], F32R)
                vp4 = cpool.tile([H, 4], F32R)
                nc.sync.dma_start(wk[:], wk_in[:])
                nc.sync.dma_start(vp4[:], vp_in[:])

            out_sb = opool.tile([1, BPC * DV], F32)

            def load_ktile(b, t):
                if hilo:
                    kht = kpool.tile([D, KTILE], BF16, tag="kht")
                    klt = kpool.tile([D, KTILE], BF16, tag="klt")
                    nc.sync.dma_start(kht[:], kh_in[b, :, t * CHUNK : t * CHUNK + KTILE])
                    nc.sync.dma_start(klt[:], kl_in[b, :, t * CHUNK : t * CHUNK + KTILE])
                    return (kht, klt)
                ktile = kpool.tile([D, KTILE], F32R, tag="kt")
                nc.sync.dma_start(ktile[:], kT_in[b, :, t * CHUNK : t * CHUNK + KTILE])
                return ktile

            for b in range(BPC):
                # v~ tiles for this batch (SWDGE queue so the large v
                # prefetches never head-of-line-block the kT stream, whose
                # issue rate is throttled by PE back-pressure)
                kts = {}
                if b == 0:
                    kts[0] = load_ktile(0, 0)
                    v_tiles = [None] * NVT
                else:
                    v_tiles = next_v_tiles
                next_v_tiles = [None] * NVT

                acc = o_pool.tile([1, DV], F32, tag="acc")
                w = wpool.tile([SUB, L // SUB], F32R, tag="w")
                for t in range(NCH):
                    if t % KCH == 0 and t // KCH not in kts:
                        kts[t // KCH] = load_ktile(b, t)
                    # batch 0 pulls its own v spread through its front half
                    if b == 0 and t % 2 == 0 and t // 2 < NVT:
                        vtile = vpool.tile([SUB, VT_COLS * DV], F32R, tag="vt")
                        nc.gpsimd.dma_start(vtile[:], v_in[0, t // 2])
                        v_tiles[t // 2] = vtile
                    # prefetch next batch's v in the BACK half of this batch,
                    # when the k lookahead buffers are already full
                    voff = NCH - 2 * NVT
                    if t >= voff and (t - voff) % 2 == 0 and b + 1 < BPC:
                        vt = (t - voff) // 2
                        vtile = vpool.tile([SUB, VT_COLS * DV], F32R, tag="vt")
                        nc.gpsimd.dma_start(vtile[:], v_in[b + 1, vt])
                        next_v_tiles[vt] = vtile

                    cs = slice((t % KCH) * CHUNK, (t % KCH + 1) * CHUNK)
                    pre = pre_pool.tile([H, CHUNK], F32, tag="pre")
                    if hilo:
                        kht, klt = kts[t // KCH]
                        nc.tensor.matmul(
                            pre[:], wkh[:], kht[:, cs], start=True, stop=False
                        )
                        nc.tensor.matmul(
                            pre[:], wkl[:], kht[:, cs], start=False, stop=False
                        )
                        nc.tensor.matmul(
                            pre[:], wkh[:], klt[:, cs], start=False, stop=True
                        )
                    else:
                        nc.tensor.matmul(
                            pre[:], wk[:], kts[t // KCH][:, cs], start=True, stop=True
                        )

                    if hilo:
                        h32 = hpool.tile([H, CHUNK], F32, tag="h32")
                        nc.scalar.activation(
                            h32[:], pre[:], ACTF.Tanh, bias=qwq[:, b : b + 1],
                            scale=1.0,
                        )
                        hh = hpool.tile([H, CHUNK], BF16, tag="hh")
                        nc.vector.tensor_copy(hh[:], h32[:])
                        hl = hpool.tile([H, CHUNK], BF16, tag="hl")
                        nc.vector.tensor_sub(hl[:], h32[:], hh[:])
                        if t % 4 == 0:
                            scol4 = s_pool.tile([SUB, 4 * NSUB], F32, tag="scol")
                        scol = scol4[:, (t % 4) * NSUB : (t % 4 + 1) * NSUB]
                        for j in range(NSUB):
                            js = slice(j * SUB, (j + 1) * SUB)
                            nc.tensor.matmul(
                                scol[:, j : j + 1], hh[:, js], vph[:],
                                start=True, stop=False,
                            )
                            nc.tensor.matmul(
                                scol[:, j : j + 1], hh[:, js], vpl[:],
                                start=False, stop=False,
                            )
                            nc.tensor.matmul(
                                scol[:, j : j + 1], hl[:, js], vph[:],
                                start=False, stop=True,
                            )
                        if t % 4 == 3:
                            nc.scalar.activation(
                                w[:, NSUB * (t - 3) : NSUB * (t + 1)],
                                scol4[:],
                                ACTF.Exp,
                            )
                    else:
                        hid = hpool.tile([H, CHUNK], F32R, tag="hid")
                        nc.scalar.activation(
                            hid[:], pre[:], ACTF.Tanh, bias=qwq[:, b : b + 1],
                            scale=1.0,
                        )
                        scol = s_pool.tile([SUB, 4 * NSUB], F32, tag="scol")
                        for j in range(NSUB):
                            nc.tensor.matmul(
                                scol[:, 4 * j : 4 * j + 4],
                                hid[:, j * SUB : (j + 1) * SUB],
                                vp4[:],
                                start=True,
                                stop=True,
                            )
                        nc.scalar.activation(
                            w[:, NSUB * t : NSUB * (t + 1)],
                            scol[:, 0 : 4 * NSUB : 4],
                            ACTF.Exp,
                        )

                nsub_total = L // SUB
                for tp in range(nsub_total):
                    vt, col = divmod(tp, VT_COLS)
                    nc.tensor.matmul(
                        acc[:],
                        w[:, tp : tp + 1],
                        v_tiles[vt][:, col * DV : (col + 1) * DV],
                        start=(tp == 0),
                        stop=(tp == nsub_total - 1),
                    )
                nc.scalar.copy(out_sb[:, b * DV : (b + 1) * DV], acc[:])

            nc.sync.dma_start(out_d[:], out_sb[:])

    _split_excess_waits(nc)
    return nc


def _prep_inputs(q, k, v, W_line, v_param, mode=MODE):
    """Host-side shard + layout prep. Returns per-core input maps."""
    hilo = mode == "hilo"
    qWq = q.astype(np.float64) @ W_line[:D].astype(np.float64)  # (B, H)
    wk = np.ascontiguousarray(W_line[D:]).astype(np.float32)  # (D, H)

    if hilo:
        wkh = np.ascontiguousarray(wk.astype(ml_dtypes.bfloat16))
        wkl = np.ascontiguousarray(
            (wk - wkh.astype(np.float32)).astype(ml_dtypes.bfloat16)
        )
        vph = v_param.astype(ml_dtypes.bfloat16)
        vpl = (v_param - vph.astype(np.float32)).astype(ml_dtypes.bfloat16)
        vpair = np.ascontiguousarray(
            np.stack([vph, vpl], axis=1)
        )  # [H, 2] bf16 -> one f32 col
    else:
        vp4 = np.tile(v_param[:, None], (1, 4)).astype(np.float32)

    in_maps = []
    for c in range(NCORES):
        bs = slice(c * BPC, (c + 1) * BPC)
        kT = np.ascontiguousarray(k[bs].transpose(0, 2, 1))  # (BPC, D, L)
        vv = np.zeros((BPC, L, DV), dtype=np.float32)
        vv[:, :, :D] = v[bs]
        vv[:, :, D] = 1.0
        # permute into the SBUF tile layout: [b][vt][p][t*DV+d]
        vv = np.ascontiguousarray(
            vv.reshape(BPC, NVT, VT_COLS, SUB, DV)
            .transpose(0, 1, 3, 2, 4)
            .reshape(BPC, NVT, SUB, VT_COLS * DV)
        )
        qwq = np.ascontiguousarray(qWq[bs].T.astype(np.float32))  # (H, BPC)
        m = {"vv": vv}
        if hilo:
            kh = kT.astype(ml_dtypes.bfloat16)
            kl = (kT - kh.astype(np.float32)).astype(ml_dtypes.bfloat16)
            cst = np.zeros((128, 133), dtype=np.float32)
            cst[:, 0:4] = qwq
            cst[:, 4:68] = wkh.view(np.float32)
            cst[:, 68:132] = wkl.view(np.float32)
            cst[:, 132:133] = vpair.view(np.float32)
            m.update(kh=kh, kl=kl, cst=cst)
        else:
            m.update(kT=kT, wk=wk, vp=vp4, qwq=qwq)
        in_maps.append(m)
    return in_maps


def _gather_output(results):
    out = np.empty((B, D), dtype=np.float32)
    for c, r in enumerate(results):
        rows = r["out"].reshape(BPC, DV).astype(np.float64)
        out[c * BPC : (c + 1) * BPC] = (rows[:, :D] / rows[:, D : D + 1]).astype(
            np.float32
        )
    return out


def run(q, k, v, W_line, v_param, trace=False, mode=MODE, **spmd_kwargs):
    from concourse.bass_utils import run_bass_kernel_spmd

    key = ("nc", mode)
    if key not in _CACHE:
        _CACHE[key] = build_nc(mode)
    nc = _CACHE[key]
    in_maps = _prep_inputs(q, k, v, W_line, v_param, mode)
    res = run_bass_kernel_spmd(
        nc, in_maps, list(range(NCORES)), trace=trace, **spmd_kwargs
    )
    return _gather_output(res.results), res


def kernel(q, k, v, W_line, v_param):
    out, _ = run(q, k, v, W_line, v_param, trace=False)
    return out



# revision 4
# speedup vs baseline: 1.5209x; 1.5209x over previous
"""Bahdanau-style additive attention on 8 TRN2 NeuronCores.

  hidden = tanh(q @ Wq + k @ Wk)        (B, L, H)
  scores = hidden @ v_param             (B, L)
  attn   = softmax(scores, axis=-1)
  out    = attn @ v                     (B, D)

Sharding: data-parallel over batch — 4 batches per core (B=32, 8 cores).

Per-core device pipeline (all-fp16 front end, bf16 back end):

  W1  pre[H, 512]  = wk16.T @ k16       single fp16 matmul per chunk
  ACT hh = tanh(pre + qWq_b) -> fp16    per-partition bias, fp16 out
  W2  scol[:, j]   = hh_j.T @ vp16      score COLUMNS, [128, 64] per batch
  ACT w = exp(scol) -> bf16, accum_out  wsum[128,1] = row sums (denominator)
  W3  acc[1, 128]  = w_col.T @ v_bf16   64 accumulating matmuls
      den[1, 1]    = ones.T @ wsum      cross-partition sum of wsum
  host: out = acc / den

Numerics (validated on host against the f64 reference for these inputs):
k=f16, wk=f16, hidden=f16, vp=f16, w=bf16, v=bf16 -> ~2.5e-3 max rel err
(vs the 2e-2 gate). fp16's 11-bit mantissa keeps score error ~1e-3; bf16 for
w is required for range (w = exp(score), scores up to ~40, no max-subtract).

DMA: k 8.39MB + v 8.39MB = 16.8MB/core (vs 34.1MB baseline).
"""

import ml_dtypes
import numpy as np

import concourse.bass as bass
import concourse.mybir as mybir
from concourse.tile import TileContext

B, L, D, H = 32, 8192, 128, 128
NCORES = 8
BPC = B // NCORES  # batches per core
CHUNK = 512  # L positions per W1/tanh chunk (psum bank limit)
NCH = L // CHUNK  # 16 chunks per batch
KTILE = 2048  # L positions per k/v DMA tile
KCH = KTILE // CHUNK  # W1 chunks per kT tile (4)
NKT = L // KTILE  # k tiles per batch (4)
SUB = 128  # L positions per W2/W3 sub-chunk (stationary width)
NSUB = CHUNK // SUB  # 4
VT_COLS = 16  # W3 sub-chunks per v SBUF tile
NVT = L // (SUB * VT_COLS)  # 4 v tiles per batch
ODV = D + 1  # out row: 128 data + denominator

F32 = mybir.dt.float32
F16 = mybir.dt.float16
BF16 = mybir.dt.bfloat16
ACTF = mybir.ActivationFunctionType
ALU = mybir.AluOpType

_CACHE = {}


def _split_excess_waits(nc, max_waits=1):
    """walrus in this env accepts at most one sync-wait per instruction;
    move extras onto InstNoOps placed just before (same engine, in order)."""
    for fn in nc.m.functions:
        for bb in fn.blocks:
            insts = list(bb.instructions)
            new_insts = []
            for ins in insts:
                si = ins.sync_info
                waits = list(si.on_wait) if si and si.on_wait else []
                if len(waits) > max_waits:
                    extra, keep = waits[:-max_waits], waits[-max_waits:]
                    for g0 in range(0, len(extra), max_waits):
                        pre = mybir.InstNoOp(
                            name=f"{ins.name}-waitsplit{g0}",
                            engine=ins.engine,
                            ins=[],
                            outs=[],
                            sync_info=mybir.SyncInfo(
                                on_wait=extra[g0 : g0 + max_waits], on_update=[]
                            ),
                        )
                        nc.register_instruction(pre, overwrite=True)
                        new_insts.append(pre)
                    ins.sync_info = mybir.SyncInfo(
                        on_wait=keep, on_update=list(si.on_update or [])
                    )
                new_insts.append(ins)
            if len(new_insts) != len(insts):
                bb.instructions[:] = new_insts


def build_nc():
    nc = bass.Bass("TRN2")

    k_in = nc.dram_tensor("k16", [BPC, D, L], F16, kind="ExternalInput")
    v_in = nc.dram_tensor("vv", [BPC, NVT, SUB, VT_COLS * D], BF16, kind="ExternalInput")
    # packed consts: cols 0:4 qwq (f32), 4:68 wk16 (fp16 pairs), 68 vp16|pad
    cst_in = nc.dram_tensor("cst", [128, 69], F32, kind="ExternalInput")
    out_d = nc.dram_tensor("out", [1, BPC * ODV], F32, kind="ExternalOutput")

    with TileContext(nc) as tc:
        with (
            tc.tile_pool(name="const", bufs=1) as cpool,
            tc.tile_pool(name="kp", bufs=4) as kpool,
            tc.tile_pool(name="vp_", bufs=2 * NVT) as vpool,
            tc.tile_pool(name="hp", bufs=4) as hpool,
            tc.tile_pool(name="wp", bufs=2) as wpool,
            tc.tile_pool(name="ob", bufs=1) as opool,
            tc.tile_pool(name="pre", bufs=2, space="PSUM") as pre_pool,
            tc.tile_pool(name="sps", bufs=2, space="PSUM") as s_pool,
            tc.tile_pool(name="ops", bufs=2, space="PSUM") as o_pool,
        ):
            # HAM warm-up on zeroed tiles: needs no DMA, so the PE clock
            # gate lifts during the Tile preamble / first k transfer.
            zwarm = cpool.tile([128, 256], BF16)
            nc.gpsimd.memset(zwarm[:], 0.0)
            warm_ps = pre_pool.tile([H, CHUNK], F32, tag="pre")
            for _ in range(8):
                nc.tensor.matmul(
                    warm_ps[:, :256], zwarm[:, :128], zwarm[:], start=True, stop=True
                )

            cst = cpool.tile([128, 69], F32)
            nc.sync.dma_start(cst[:], cst_in[:])
            qwq = cst[:, 0:4]
            wk = cst[:, 4:68].bitcast(F16)
            vp = cst[:, 68:69].bitcast(F16)[:, 0:1]
            ones = cpool.tile([128, 1], F32)
            nc.gpsimd.memset(ones[:], 1.0)

            out_sb = opool.tile([1, BPC * ODV], F32)

            def load_ktile(b, i):
                kt = kpool.tile([D, KTILE], F16, tag="kt")
                nc.sync.dma_start(kt[:], k_in[b, :, i * KTILE : (i + 1) * KTILE])
                return kt

            kts = {(0, 0): load_ktile(0, 0), (0, 1): load_ktile(0, 1)}
            v_tiles = {}

            for b in range(BPC):
                acc = o_pool.tile([1, ODV], F32, tag="acc")
                scol = s_pool.tile([SUB, L // SUB], F32, tag="scol")
                w = wpool.tile([SUB, L // SUB], BF16, tag="w")
                wsum = wpool.tile([SUB, 1], F32, tag="wsum")

                for t in range(NCH):
                    # prefetch k two tiles ahead (wraps into the next batch)
                    if t % KCH == 0:
                        gi = t // KCH + 2
                        tgt = (b, gi) if gi < NKT else (b + 1, gi - NKT)
                        if tgt[0] < BPC and tgt not in kts:
                            kts[tgt] = load_ktile(*tgt)
                    # v tiles: batch 0 pulls its own spread through its front
                    # half; later batches were prefetched in the previous
                    # batch's back half (SWDGE queue so the big v transfers
                    # never head-of-line-block the k stream).
                    if b == 0 and t % 2 == 0 and t // 2 < NVT:
                        vt = vpool.tile([SUB, VT_COLS * D], BF16, tag="vt")
                        nc.gpsimd.dma_start(vt[:], v_in[0, t // 2])
                        v_tiles[(0, t // 2)] = vt
                    voff = NCH - 2 * NVT
                    if t >= voff and (t - voff) % 2 == 0 and b + 1 < BPC:
                        vi = (t - voff) // 2
                        vt = vpool.tile([SUB, VT_COLS * D], BF16, tag="vt")
                        nc.gpsimd.dma_start(vt[:], v_in[b + 1, vi])
                        v_tiles[(b + 1, vi)] = vt

                    cs = slice((t % KCH) * CHUNK, (t % KCH + 1) * CHUNK)
                    pre = pre_pool.tile([H, CHUNK], F32, tag="pre")
                    nc.tensor.matmul(
                        pre[:], wk[:], kts[(b, t // KCH)][:, cs], start=True, stop=True
                    )

                    hh = hpool.tile([H, CHUNK], F16, tag="hh")
                    nc.scalar.activation(
                        hh[:], pre[:], ACTF.Tanh, bias=qwq[:, b : b + 1], scale=1.0
                    )
                    for j in range(NSUB):
                        js = slice(j * SUB, (j + 1) * SUB)
                        nc.tensor.matmul(
                            scol[:, t * NSUB + j : t * NSUB + j + 1],
                            hh[:, js],
                            vp[:],
                            start=True,
                            stop=True,
                        )

                # softmax weights + free denominator (row sums via accum_out)
                nc.scalar.activation(w[:], scol[:], ACTF.Exp, accum_out=wsum[:])
                nc.tensor.matmul(
                    acc[:, D : D + 1], ones[:], wsum[:], start=True, stop=True
                )

                nsub_total = L // SUB
                for tp in range(nsub_total):
                    vt, col = divmod(tp, VT_COLS)
                    nc.tensor.matmul(
                        acc[:, 0:D],
                        w[:, tp : tp + 1],
                        v_tiles[(b, vt)][:, col * D : (col + 1) * D],
                        start=(tp == 0),
                        stop=(tp == nsub_total - 1),
                    )
                nc.scalar.copy(out_sb[:, b * ODV : (b + 1) * ODV], acc[:])
                for vt in range(NVT):
                    v_tiles.pop((b, vt), None)

            nc.sync.dma_start(out_d[:], out_sb[:])

    _split_excess_waits(nc)
    return nc


def _prep_inputs(q, k, v, W_line, v_param):
    """Host-side shard + layout prep. Returns per-core input maps."""
    qWq = q.astype(np.float64) @ W_line[:D].astype(np.float64)  # (B, H)
    wk16 = np.ascontiguousarray(W_line[D:]).astype(np.float16)  # (D, H)
    vp16 = np.zeros((H, 2), dtype=np.float16)
    vp16[:, 0] = v_param.astype(np.float16)

    cst_base = np.zeros((128, 69), dtype=np.float32)
    cst_base[:, 4:68] = wk16.view(np.float32)
    cst_base[:, 68:69] = vp16.view(np.float32)

    in_maps = []
    for c in range(NCORES):
        bs = slice(c * BPC, (c + 1) * BPC)
        k16 = np.ascontiguousarray(k[bs].transpose(0, 2, 1)).astype(np.float16)
        # v into the SBUF tile layout: [b][vt][p][col*D+d], bf16
        vv = np.ascontiguousarray(
            v[bs]
            .reshape(BPC, NVT, VT_COLS, SUB, D)
            .transpose(0, 1, 3, 2, 4)
            .reshape(BPC, NVT, SUB, VT_COLS * D)
        ).astype(ml_dtypes.bfloat16)
        cst = cst_base.copy()
        cst[:, 0:4] = qWq[bs].T.astype(np.float32)
        in_maps.append({"k16": k16, "vv": vv, "cst": cst})
    return in_maps


def _gather_output(results):
    out = np.empty((B, D), dtype=np.float32)
    for c, r in enumerate(results):
        rows = r["out"].reshape(BPC, ODV).astype(np.float64)
        out[c * BPC : (c + 1) * BPC] = (rows[:, :D] / rows[:, D : D + 1]).astype(
            np.float32
        )
    return out


def run(q, k, v, W_line, v_param, trace=False, **spmd_kwargs):
    from concourse.bass_utils import run_bass_kernel_spmd

    if "nc" not in _CACHE:
        _CACHE["nc"] = build_nc()
    nc = _CACHE["nc"]
    in_maps = _prep_inputs(q, k, v, W_line, v_param)
    res = run_bass_kernel_spmd(
        nc, in_maps, list(range(NCORES)), trace=trace, **spmd_kwargs
    )
    return _gather_output(res.results), res


def kernel(q, k, v, W_line, v_param):
    out, _ = run(q, k, v, W_line, v_param, trace=False)
    return out


# revision 6
# speedup vs baseline: 1.7533x; 1.1528x over previous
"""Bahdanau-style additive attention on 8 TRN2 NeuronCores.

  hidden = tanh(q @ Wq + k @ Wk)        (B, L, H)
  scores = hidden @ v_param             (B, L)
  attn   = softmax(scores, axis=-1)
  out    = attn @ v                     (B, D)

Sharding: data-parallel over batch — 4 batches per core (B=32, 8 cores).

Per-core pipeline, software-pipelined in PAIR slots (1024 positions):

  W1  pre[H, 1024] = wk16.T @ k16        2 fp16 matmuls (one per psum bank)
  ACT hh = tanh(pre + qWq_b) -> fp16     1024-wide, per-partition bias
  W2  scol[:, j]   = hh_j.T @ vp16       8 score-column matmuls (lags W1 by 1)
  ACT w = exp(scol_pair) -> bf16         8 cols; accum_out -> wsum column
  W3  acc[D, 1]   += v_j.T @ w_col       v STATIONARY (weight-load streams 4
                                         cols/cy, so this is 4x faster than
                                         w-stationary / v-moving), lags by 2
      den[1, 8]    = ones.T @ wsum       per-batch; host sums the 8 parts
  host: out = acc / den

Numerics (validated on host against the f64 reference for these inputs):
k=f16, wk=f16, hidden=f16, vp=f16, w=bf16, v=bf16 -> ~2.6e-3 max rel err
(vs the 2e-2 gate). fp16's 11-bit mantissa keeps score error ~1e-3; bf16 for
w is required for range (w = exp(score), scores up to ~40, no max-subtract).

DMA: k 8.39MB + v 8.39MB = 16.8MB/core at ~325 B/ns -> ~52us floor.
"""

import ml_dtypes
import numpy as np

import concourse.bass as bass
import concourse.mybir as mybir
from concourse.tile import TileContext

B, L, D, H = 32, 8192, 128, 128
NCORES = 8
BPC = B // NCORES  # batches per core
CHUNK = 512  # L positions per W1 matmul (psum bank limit)
PAIR = 2 * CHUNK  # positions per pipeline slot
NP_B = L // PAIR  # pair slots per batch (8)
NSLOT = BPC * NP_B  # total slots (32)
KTILE = 2048  # L positions per k/v DMA tile
NKT = L // KTILE  # k tiles per batch (4)
SUB = 128  # L positions per W2/W3 sub-chunk (stationary width)
VT_COLS = 16  # W3 sub-chunks per v SBUF tile
NVT = L // (SUB * VT_COLS)  # v tiles per batch (4)
ODV = 1 + NP_B  # out cols per batch: acc col + 8 denominator parts

F32 = mybir.dt.float32
F16 = mybir.dt.float16
BF16 = mybir.dt.bfloat16
ACTF = mybir.ActivationFunctionType

_CACHE = {}


def _split_excess_waits(nc, max_waits=1):
    """walrus in this env accepts at most one sync-wait per instruction;
    move extras onto InstNoOps placed just before (same engine, in order)."""
    for fn in nc.m.functions:
        for bb in fn.blocks:
            insts = list(bb.instructions)
            new_insts = []
            for ins in insts:
                si = ins.sync_info
                waits = list(si.on_wait) if si and si.on_wait else []
                if len(waits) > max_waits:
                    extra, keep = waits[:-max_waits], waits[-max_waits:]
                    for g0 in range(0, len(extra), max_waits):
                        pre = mybir.InstNoOp(
                            name=f"{ins.name}-waitsplit{g0}",
                            engine=ins.engine,
                            ins=[],
                            outs=[],
                            sync_info=mybir.SyncInfo(
                                on_wait=extra[g0 : g0 + max_waits], on_update=[]
                            ),
                        )
                        nc.register_instruction(pre, overwrite=True)
                        new_insts.append(pre)
                    ins.sync_info = mybir.SyncInfo(
                        on_wait=keep, on_update=list(si.on_update or [])
                    )
                new_insts.append(ins)
            if len(new_insts) != len(insts):
                bb.instructions[:] = new_insts


def build_nc():
    nc = bass.Bass("TRN2")

    k_in = nc.dram_tensor("k16", [BPC, D, L], F16, kind="ExternalInput")
    v_in = nc.dram_tensor("vv", [BPC, NVT, SUB, VT_COLS * D], BF16, kind="ExternalInput")
    # packed consts: cols 0:4 qwq (f32), 4:68 wk16 (fp16 pairs), 68 vp16|pad
    cst_in = nc.dram_tensor("cst", [128, 69], F32, kind="ExternalInput")
    out_d = nc.dram_tensor("out", [128, BPC * ODV], F32, kind="ExternalOutput")

    with TileContext(nc) as tc:
        with (
            tc.tile_pool(name="const", bufs=1) as cpool,
            tc.tile_pool(name="kp", bufs=6) as kpool,
            tc.tile_pool(name="vp_", bufs=2 * NVT + 1) as vpool,
            tc.tile_pool(name="hp", bufs=3) as hpool,
            tc.tile_pool(name="wp", bufs=2) as wpool,
            tc.tile_pool(name="ob", bufs=1) as opool,
            tc.tile_pool(name="pre", bufs=2, space="PSUM") as pre_pool,
            tc.tile_pool(name="sps", bufs=2, space="PSUM") as s_pool,
            tc.tile_pool(name="ops", bufs=2, space="PSUM") as o_pool,
        ):
            # HAM warm-up on zeroed tiles: needs no DMA, so the PE clock
            # gate lifts during the Tile preamble / first k transfer.
            zwarm = cpool.tile([128, 256], BF16)
            nc.gpsimd.memset(zwarm[:], 0.0)
            warm_ps = pre_pool.tile([H, PAIR], F32, tag="pre")
            for _ in range(8):
                nc.tensor.matmul(
                    warm_ps[:, :256], zwarm[:, :128], zwarm[:], start=True, stop=True
                )
            # dummy activation: pulls the ACT function table load (~1.3us)
            # off the critical path, concurrent with the first k transfer
            tdum = cpool.tile([128, 1], F32)
            nc.scalar.activation(tdum[:], zwarm[:, 0:1], ACTF.Tanh)

            cst = cpool.tile([128, 69], F32)
            nc.sync.dma_start(cst[:], cst_in[:])
            qwq = cst[:, 0:4]
            wk = cst[:, 4:68].bitcast(F16)
            vp = cst[:, 68:69].bitcast(F16)[:, 0:1]
            ones = cpool.tile([128, 1], F32)
            nc.gpsimd.memset(ones[:], 1.0)

            out_sb = opool.tile([128, BPC * ODV], F32)

            # k tile G (global, 0..15) covers pair slots 2G, 2G+1
            kts = {}

            def load_ktile(g, split=1):
                b, i = divmod(g, NKT)
                kt = kpool.tile([D, KTILE], F16, tag="kt", name="kt")
                w_ = KTILE // split
                for s in range(split):
                    nc.sync.dma_start(
                        kt[:, s * w_ : (s + 1) * w_],
                        k_in[b, :, i * KTILE + s * w_ : i * KTILE + (s + 1) * w_],
                    )
                kts[g] = kt

            # first tile chunk-granular so W1 slot 0 starts after ~128KB
            load_ktile(0, split=4)
            load_ktile(1)

            v_tiles = {}

            def load_vtile(b, vt):
                t = vpool.tile([SUB, VT_COLS * D], BF16, tag="vt", name="vt")
                nc.gpsimd.dma_start(t[:], v_in[b, vt])
                v_tiles[(b, vt)] = t

            scols, ws, wsums, accs = {}, {}, {}, {}

            def w2_block(P):
                b, p = divmod(P, NP_B)
                if p == 0:
                    scols[b] = s_pool.tile([SUB, L // SUB], F32, tag="scol", name="scol")
                    ws[b] = wpool.tile([SUB, L // SUB], BF16, tag="w", name="w")
                    wsums[b] = wpool.tile([SUB, NP_B], F32, tag="wsum", name="wsum")
                hh = hhs.pop(P)
                for j in range(PAIR // SUB):
                    c = p * (PAIR // SUB) + j
                    nc.tensor.matmul(
                        scols[b][:, c : c + 1],
                        hh[:, j * SUB : (j + 1) * SUB],
                        vp[:],
                        start=True,
                        stop=True,
                    )
                # softmax weights for this pair + its denominator part
                nc.scalar.activation(
                    ws[b][:, p * 8 : (p + 1) * 8],
                    scols[b][:, p * 8 : (p + 1) * 8],
                    ACTF.Exp,
                    accum_out=wsums[b][:, p : p + 1],
                )

            def w3_block(P):
                b, p = divmod(P, NP_B)
                if p == 0:
                    accs[b] = o_pool.tile([128, ODV], F32, tag="acc", name="acc")
                for j in range(PAIR // SUB):
                    c = p * (PAIR // SUB) + j
                    vt, col = divmod(c, VT_COLS)
                    nc.tensor.matmul(
                        accs[b][:, 0:1],
                        v_tiles[(b, vt)][:, col * D : (col + 1) * D],
                        ws[b][:, c : c + 1],
                        start=(c == 0),
                        stop=(c == L // SUB - 1),
                    )
                if p == NP_B - 1:
                    nc.tensor.matmul(
                        accs[b][0:1, 1:ODV], ones[:], wsums[b][:], start=True, stop=True
                    )
                    nc.scalar.copy(
                        out_sb[:, b * ODV : (b + 1) * ODV], accs[b][:]
                    )
                    for vt in range(NVT):
                        v_tiles.pop((b, vt), None)

            hhs = {}
            for P in range(NSLOT):
                b, p = divmod(P, NP_B)
                # k prefetch: tile P//2+2, two tiles (~3us) ahead
                if P % 2 == 0 and P // 2 + 2 < 2 * NKT * BPC // 2:
                    g = P // 2 + 2
                    if g < NKT * BPC and g not in kts:
                        load_ktile(g)
                # v prefetch: batch 0 pulls its own early; later batches were
                # loaded one batch ahead (SWDGE queue, never blocks k stream)
                if b == 0 and 1 <= P <= NVT:
                    load_vtile(0, P - 1)
                if p in (1, 3, 5, 7) and b + 1 < BPC:
                    load_vtile(b + 1, (p - 1) // 2)

                # W1 for this pair: two 512-wide matmuls into one psum tile
                g = P // 2
                cs0 = (P % 2) * PAIR
                pre = pre_pool.tile([H, PAIR], F32, tag="pre")
                for h in range(2):
                    nc.tensor.matmul(
                        pre[:, h * CHUNK : (h + 1) * CHUNK],
                        wk[:],
                        kts[g][:, cs0 + h * CHUNK : cs0 + (h + 1) * CHUNK],
                        start=True,
                        stop=True,
                    )
                hh = hpool.tile([H, PAIR], F16, tag="hh", name="hh")
                nc.scalar.activation(
                    hh[:], pre[:], ACTF.Tanh, bias=qwq[:, b : b + 1], scale=1.0
                )
                hhs[P] = hh

                if P >= 1:
                    w2_block(P - 1)
                if P >= 2:
                    w3_block(P - 2)

            w2_block(NSLOT - 1)
            w3_block(NSLOT - 2)
            w3_block(NSLOT - 1)

            nc.sync.dma_start(out_d[:], out_sb[:])

    _split_excess_waits(nc)
    return nc


def _prep_inputs(q, k, v, W_line, v_param):
    """Host-side shard + layout prep. Returns per-core input maps."""
    qWq = q.astype(np.float64) @ W_line[:D].astype(np.float64)  # (B, H)
    wk16 = np.ascontiguousarray(W_line[D:]).astype(np.float16)  # (D, H)
    vp16 = np.zeros((H, 2), dtype=np.float16)
    vp16[:, 0] = v_param.astype(np.float16)

    cst_base = np.zeros((128, 69), dtype=np.float32)
    cst_base[:, 4:68] = wk16.view(np.float32)
    cst_base[:, 68:69] = vp16.view(np.float32)

    in_maps = []
    for c in range(NCORES):
        bs = slice(c * BPC, (c + 1) * BPC)
        k16 = np.ascontiguousarray(k[bs].transpose(0, 2, 1)).astype(np.float16)
        # v into the SBUF tile layout: [b][vt][p][col*D+d], bf16
        vv = np.ascontiguousarray(
            v[bs]
            .reshape(BPC, NVT, VT_COLS, SUB, D)
            .transpose(0, 1, 3, 2, 4)
            .reshape(BPC, NVT, SUB, VT_COLS * D)
        ).astype(ml_dtypes.bfloat16)
        cst = cst_base.copy()
        cst[:, 0:4] = qWq[bs].T.astype(np.float32)
        in_maps.append({"k16": k16, "vv": vv, "cst": cst})
    return in_maps


def _gather_output(results):
    out = np.empty((B, D), dtype=np.float32)
    for c, r in enumerate(results):
        cols = r["out"].astype(np.float64)  # [128, BPC*ODV]
        for b in range(BPC):
            acc = cols[:, b * ODV]
            den = cols[0, b * ODV + 1 : (b + 1) * ODV].sum()
            out[c * BPC + b] = (acc / den).astype(np.float32)
    return out


def run(q, k, v, W_line, v_param, trace=False, **spmd_kwargs):
    from concourse.bass_utils import run_bass_kernel_spmd

    if "nc" not in _CACHE:
        _CACHE["nc"] = build_nc()
    nc = _CACHE["nc"]
    in_maps = _prep_inputs(q, k, v, W_line, v_param)
    res = run_bass_kernel_spmd(
        nc, in_maps, list(range(NCORES)), trace=trace, **spmd_kwargs
    )
    return _gather_output(res.results), res


def kernel(q, k, v, W_line, v_param):
    out, _ = run(q, k, v, W_line, v_param, trace=False)
    return out
